# revision 1
# baseline (speedup 1.0000x reference)
"""GGNN (gated graph NN) message-passing kernel for 8 Trainium2 NeuronCores.

Sharding: edge-type sharding. Core c owns edge-type block c of the adjacency
matrix (columns c*N..(c+1)*N of the [N, 2E*N] adjacency, pre-transposed on the
host) plus the node shard c for the GRU update.

Per step, on core c:
  stage1: t_c = h @ W_prop[c]                      [N, D]   (h^T streamed as lhsT)
  stage2: partial_a_c = A_cT.T @ t_c               [N, D]   (A_cT resident uint8)
  RS:     a_shard = ReduceScatter_add(partial_a)   [N/8, D] (split in 2 halves so
          the first RS overlaps the second half of stage2)
  GRU:    h_shard' = GRU(a_shard, h_shard)         (transposed layout, fp32r mm)
  AG:     h^T' = AllGather(h_shard'^T)             (fp32r)

Each core's node shard is blocks {128c..128c+127, 1024+128c..1024+128c+127}
(the blocks the two half-ReduceScatters deliver to rank c).

Numerics: matmuls in float32r (fp32 with 12-bit mantissa, full PE rate at
free-dim>=256); adjacency stored as uint8 (exact for 0/1) upconverted to fp32r
on DVE; accumulation fp32 in PSUM; elementwise GRU update in fp32.
"""
import sys
if "/opt/trn_rl_repo" not in sys.path:
    sys.path.insert(0, "/opt/trn_rl_repo")

import numpy as np
import ml_dtypes

NC_CORES = 8
N = 2048          # nodes
D = 512           # state dim
ANN = 256         # annotation dim
STEPS = 5
SH = N // NC_CORES   # 256 nodes per shard
KT = D // 128        # 4
MT = N // 128        # 16


def _q12(x):
    """Round fp32 to 12 explicit mantissa bits (fp32r grid), RNE."""
    mant, ex = np.frexp(np.asarray(x, np.float32).astype(np.float64))
    return (np.round(mant * 4096) / 4096 * np.exp2(ex)).astype(np.float32)


def build(repeats=1, ablate=()):
    import concourse.bacc as bacc
    import concourse.mybir as mybir
    import concourse.tile as tile
    from concourse.masks import make_identity

    dt = mybir.dt
    nc = bacc.Bacc()
    at_p = nc.declare_dram_parameter("at", [N, N], dt.uint8, isOutput=False)
    h0t_p = nc.declare_dram_parameter("h0t", [NC_CORES * D, SH], dt.float32r,
                                      isOutput=False)
    h0sr_p = nc.declare_dram_parameter("h0sr", [D, SH], dt.float32r, isOutput=False)
    h0s_p = nc.declare_dram_parameter("h0s", [D, SH], dt.float32, isOutput=False)
    wc_p = nc.declare_dram_parameter("wc", [D, D], dt.float32r, isOutput=False)
    gw_p = nc.declare_dram_parameter("gw", [6, D, D], dt.float32r, isOutput=False)
    bpc_p = nc.declare_dram_parameter("bpc", [1, D], dt.float32, isOutput=False)
    bz_p = nc.declare_dram_parameter("bzc", [D, 1], dt.float32, isOutput=False)
    br_p = nc.declare_dram_parameter("brc", [D, 1], dt.float32, isOutput=False)
    bh_p = nc.declare_dram_parameter("bhc", [D, 1], dt.float32, isOutput=False)
    out_p = nc.declare_dram_parameter("out", [D, SH], dt.float32, isOutput=True)
    RG = [list(range(NC_CORES))]

    from contextlib import ExitStack
    with tile.TileContext(nc) as tc, ExitStack() as stk:
        res = stk.enter_context(tc.tile_pool(name="res", bufs=1))
        p_mm = stk.enter_context(tc.tile_pool(name="pmm", bufs=8, space="PSUM"))
        p_hc = stk.enter_context(tc.tile_pool(name="phc", bufs=6))
        p_t = stk.enter_context(tc.tile_pool(name="pt", bufs=1))
        p_ar = stk.enter_context(tc.tile_pool(name="par", bufs=3))
        p_asb = stk.enter_context(tc.tile_pool(name="pasb", bufs=2))
        p_sm = stk.enter_context(tc.tile_pool(name="psm", bufs=1))
        p_h = stk.enter_context(tc.tile_pool(name="ph", bufs=2))
        dram = stk.enter_context(tc.tile_pool(name="dram", bufs=2, space="DRAM"))

        # ---- setup: constants, weights, adjacency ----
        identity = res.tile([128, 128], dt.float32, tag="identity")
        make_identity(nc, identity[:])
        ones = res.tile([1, 128], dt.float32, tag="ones")
        nc.vector.memset(ones[:], 1.0)
        bpc_t = res.tile([1, D], dt.float32, tag="bpc")
        nc.sync.dma_start(bpc_t[:], bpc_p[:])
        pb = p_mm.tile([128, D], dt.float32, tag="mm")
        nc.tensor.matmul(pb[:], ones[:], bpc_t[:], start=True, stop=True)
        bias_bcast = res.tile([128, D], dt.float32, tag="bias_bcast")
        nc.vector.tensor_copy(bias_bcast[:], pb[:])

        bias_tiles = {}
        for nm, par in (("z", bz_p), ("r", br_p), ("h", bh_p)):
            for f in range(KT):
                bt = res.tile([128, 1], dt.float32, tag=f"b{nm}{f}")
                nc.sync.dma_start(bt[:], par[f * 128:(f + 1) * 128, :])
                bias_tiles[(nm, f)] = bt

        wc_t = []
        for k in range(KT):
            w = res.tile([128, D], dt.float32r, tag=f"wc{k}")
            nc.sync.dma_start(w[:], wc_p[k * 128:(k + 1) * 128, :])
            wc_t.append(w)

        at_t = []
        for m in range(MT):
            a = res.tile([128, N], dt.uint8, tag=f"at{m}")
            nc.sync.dma_start(a[:], at_p[m * 128:(m + 1) * 128, :])
            at_t.append(a)

        # resident GRU weights (fp32r), loaded once
        gw_res = []
        for g in range(6):
            w = res.tile([128, KT, D], dt.float32r, tag=f"gwr{g}")
            nc.scalar.dma_start(w[:], gw_p[g].rearrange("(k p) f -> p k f", p=128))
            gw_res.append(w)

        for rep in range(repeats):
          # step-0 h state
          hsh_prev = []   # h^T shard, fp32r (GRU rhs)
          h32_prev = []   # h^T shard, fp32 (elementwise state)
          for k in range(KT):
            hr = p_h.tile([128, SH], dt.float32r, tag=f"hnr{k}")
            nc.sync.dma_start(hr[:], h0sr_p[k * 128:(k + 1) * 128, :])
            hsh_prev.append(hr)
            h3 = p_h.tile([128, SH], dt.float32, tag=f"h32{k}")
            nc.sync.dma_start(h3[:], h0s_p[k * 128:(k + 1) * 128, :])
            h32_prev.append(h3)

          ag_out_prev = None

          for s in range(STEPS):
             # ---- stage 1: t = h @ W_c  (+ b_c via broadcast add on cast) ----
             # shard layout: core cp owns node blocks {128cp, 1024+128cp}
             t_tiles = [None] * MT
             for mp in range(MT // 2):
                 if "s1" not in ablate:
                     hc = p_hc.tile([128, KT, 2, 128], dt.float32r, tag="hc")
                     blk = (h0t_p if s == 0 else ag_out_prev)[512 * mp:512 * (mp + 1), :]
                     nc.sync.dma_start(
                         hc[:], blk.rearrange("(k p) mj -> p k mj", p=128))
                 for mloc in range(2):
                     m = mp + 8 * mloc
                     pt = p_mm.tile([128, D], dt.float32, tag="mm")
                     if "s1" in ablate:
                         nc.tensor.matmul(pt[:], wc_t[0][:, 0:128], wc_t[1][:],
                                          start=True, stop=True)
                     else:
                         for k in range(KT):
                             nc.tensor.matmul(pt[:], hc[:, k, mloc, :], wc_t[k][:],
                                              start=(k == 0), stop=(k == KT - 1))
                     tm = p_t.tile([128, D], dt.float32r, tag=f"t{m}")
                     nc.vector.tensor_add(tm[:], pt[:], bias_bcast[:])
                     t_tiles[m] = tm

             # ---- stage 2: partial_a = A_cT.T @ t; RS per half (overlapped) ----
             rs_outs = []
             for grp in range(2):
                 rs_in = dram.tile([N // 2, D], dt.float32, tag=f"rs_in{grp}",
                                   name=f"rs_in{grp}")
                 pas = [p_mm.tile([128, D], dt.float32, tag="mm", name=f"pa{grp}_{i}")
                        for i in range(8)]
                 if "s2" in ablate:
                     for i in range(8):
                         nc.tensor.matmul(pas[i][:], t_tiles[0][:, 0:128],
                                          t_tiles[1][:], start=True, stop=True)
                 else:
                  for m in range(MT):
                     ar = p_ar.tile([128, 1024], dt.float32r, tag="ar")
                     nc.vector.tensor_copy(ar[:], at_t[m][:, grp * 1024:(grp + 1) * 1024])
                     for i in range(8):
                         nc.tensor.matmul(pas[i][:], ar[:, i * 128:(i + 1) * 128],
                                          t_tiles[m][:],
                                          start=(m == 0), stop=(m == MT - 1))
                 for i in range(8):
                     n = grp * 8 + i
                     asb = p_asb.tile([128, D], dt.float32, tag="asb")
                     if i % 2 == 0:
                         nc.scalar.copy(asb[:], pas[i][:])
                     else:
                         nc.vector.tensor_copy(asb[:], pas[i][:])
                     eng = nc.sync if i % 2 == 0 else nc.scalar
                     eng.dma_start(rs_in[i * 128:(i + 1) * 128, :], asb[:])
                 # RS of this half: core c receives node block grp*1024 + 128c
                 rs_out = dram.tile([128, D], dt.float32, tag=f"rs_out{grp}",
                                    name=f"rs_out{grp}")
                 if "cc" in ablate or "rs" in ablate:
                     nc.sync.dma_start(rs_out[:], rs_in[0:128, :])
                 else:
                     nc.gpsimd.collective_compute(
                         "ReduceScatter", mybir.AluOpType.add, replica_groups=RG,
                         ins=[rs_in[:]], outs=[rs_out[:]])
                 rs_outs.append(rs_out)

             # ---- transpose a_shard -> aT [D, SH] fp32r ----
             # r2=0 chunks (from RS1) transpose while RS2 is still in flight
             an_tiles = []
             for r2 in range(2):
                 an = p_sm.tile([128, D], dt.float32, tag=f"an{r2}")
                 nc.sync.dma_start(an[:], rs_outs[r2][:])
                 an_tiles.append(an)
             aT = []
             for kb in range(KT):
                 a_kb = p_sm.tile([128, SH], dt.float32r, tag=f"aT{kb}")
                 aT.append(a_kb)
             for r2 in range(2):
                 for kb in range(KT):
                     ptr = p_mm.tile([128, 128], dt.float32, tag="mm")
                     nc.tensor.transpose(ptr[:], an_tiles[r2][:, kb * 128:(kb + 1) * 128],
                                         identity[:])
                     nc.vector.tensor_copy(aT[kb][:, r2 * 128:(r2 + 1) * 128], ptr[:])

             # ---- GRU gates (transposed layout [D, SH]) ----
             def gate_mm(widx, uidx, rhs_u, func, bias_nm, out_dtype=dt.float32):
                 Wq, Uq = gw_res[widx], gw_res[uidx]
                 outs = []
                 for f in range(KT):
                     pg = p_mm.tile([128, SH], dt.float32, tag="mm")
                     if "gru" in ablate:
                         nc.tensor.matmul(pg[:], aT[0][:, 0:128], aT[0][:],
                                          start=True, stop=True)
                         nc.tensor.matmul(pg[:], rhs_u[0][:, 0:128], rhs_u[0][:],
                                          start=False, stop=True)
                         k = None
                     else:
                      for k in range(KT):
                         nc.tensor.matmul(pg[:], Wq[:, k, f * 128:(f + 1) * 128],
                                          aT[k][:], start=(k == 0), stop=False)
                      for k in range(KT):
                         nc.tensor.matmul(pg[:], Uq[:, k, f * 128:(f + 1) * 128],
                                          rhs_u[k][:], start=False, stop=(k == KT - 1))
                     og = p_sm.tile([128, SH], out_dtype, tag=f"g{bias_nm}{f}")
                     nc.scalar.activation(og[:], pg[:], func,
                                          bias=bias_tiles[(bias_nm, f)][:])
                     outs.append(og)
                 return outs

             import concourse.mybir as _mb
             if "gru" in ablate:
                 z_t = gate_mm(0, 1, hsh_prev, _mb.ActivationFunctionType.Sigmoid, "z")
                 r_t = gate_mm(2, 3, hsh_prev, _mb.ActivationFunctionType.Sigmoid, "r")
             else:
                 # z and r gates with both U-halves emitted first: the U-term
                 # matmuls depend only on local h and run while RS2 is in flight
                 pz = [p_mm.tile([128, SH], dt.float32, tag="mm", name=f"pz{f}")
                       for f in range(KT)]
                 pr = [p_mm.tile([128, SH], dt.float32, tag="mm", name=f"pr{f}")
                       for f in range(KT)]
                 for pg_l, uidx in ((pz, 1), (pr, 3)):
                     Uq = gw_res[uidx]
                     for f in range(KT):
                         for k in range(KT):
                             nc.tensor.matmul(pg_l[f][:],
                                              Uq[:, k, f * 128:(f + 1) * 128],
                                              hsh_prev[k][:],
                                              start=(k == 0), stop=False)
                 for pg_l, widx in ((pz, 0), (pr, 2)):
                     Wq = gw_res[widx]
                     for f in range(KT):
                         for k in range(KT):
                             nc.tensor.matmul(pg_l[f][:],
                                              Wq[:, k, f * 128:(f + 1) * 128],
                                              aT[k][:],
                                              start=False, stop=(k == KT - 1))
                 z_t, r_t = [], []
                 for outs, pg_l, nm, fn in (
                         (z_t, pz, "z", _mb.ActivationFunctionType.Sigmoid),
                         (r_t, pr, "r", _mb.ActivationFunctionType.Sigmoid)):
                     for f in range(KT):
                         og = p_sm.tile([128, SH], dt.float32, tag=f"g{nm}{f}",
                                        name=f"g{nm}{f}")
                         nc.scalar.activation(og[:], pg_l[f][:], fn,
                                              bias=bias_tiles[(nm, f)][:])
                         outs.append(og)
             rh = []
             for k in range(KT):
                 rhk = p_sm.tile([128, SH], dt.float32r, tag=f"rh{k}")
                 nc.vector.tensor_mul(rhk[:], r_t[k][:], h32_prev[k][:])
                 rh.append(rhk)
             ht_t = gate_mm(4, 5, rh, _mb.ActivationFunctionType.Tanh, "h")

             # ---- h' = h + z * (ht - h) ----
             hsh_new, h32_new = [], []
             last = (s == STEPS - 1)
             if not last:
                 ag_in = dram.tile([D, SH], dt.float32r, tag="ag_in")
             for k in range(KT):
                 s1 = p_sm.tile([128, SH], dt.float32, tag="gsA")
                 nc.vector.tensor_sub(s1[:], ht_t[k][:], h32_prev[k][:])
                 s2 = p_sm.tile([128, SH], dt.float32, tag="gsB")
                 nc.vector.tensor_mul(s2[:], z_t[k][:], s1[:])
                 h3 = p_h.tile([128, SH], dt.float32, tag=f"h32{k}")
                 nc.vector.tensor_add(h3[:], h32_prev[k][:], s2[:])
                 h32_new.append(h3)
                 if last:
                     nc.sync.dma_start(out_p[k * 128:(k + 1) * 128, :], h3[:])
                 else:
                     hr = p_h.tile([128, SH], dt.float32r, tag=f"hnr{k}")
                     nc.vector.tensor_copy(hr[:], h3[:])
                     hsh_new.append(hr)
                     nc.sync.dma_start(ag_in[k * 128:(k + 1) * 128, :], hr[:])

             if not last:
                 ag_out = dram.tile([NC_CORES * D, SH], dt.float32r, tag="ag_out",
                                    addr_space="Shared")
                 if "cc" in ablate or "ag" in ablate:
                     nc.sync.dma_start(ag_out[0:D, :], ag_in[:])
                 else:
                     nc.gpsimd.collective_compute(
                         "AllGather", mybir.AluOpType.bypass, replica_groups=RG,
                         ins=[ag_in[:]], outs=[ag_out[:]])
                 ag_out_prev = ag_out
                 hsh_prev, h32_prev = hsh_new, h32_new

    nc.finalize()
    return nc


_BUILT = None
TRACE = False
LAST_RESULT = None


_BUILT_R = {}


def _get_built(repeats=1, ablate=()):
    global _BUILT
    key = (repeats, tuple(ablate))
    if key != (1, ()):
        if key not in _BUILT_R:
            _BUILT_R[key] = build(repeats, ablate)
        return _BUILT_R[key]
    if _BUILT is None:
        _BUILT = build()
    return _BUILT


def prepare_in_maps(adjacency, annotations, W_prop, b_prop, Wz, Uz, bz,
                    Wr, Ur, br, Wh, Uh, bh):
    A = np.asarray(adjacency, np.float32)
    ann = np.asarray(annotations, np.float32)
    W_prop = np.asarray(W_prop, np.float32)
    b_prop = np.asarray(b_prop, np.float32)
    gw_all = _q12(np.stack([np.asarray(x, np.float32)
                            for x in (Wz, Uz, Wr, Ur, Wh, Uh)]))
    bz = np.asarray(bz, np.float32).reshape(D, 1)
    br = np.asarray(br, np.float32).reshape(D, 1)
    bh = np.asarray(bh, np.float32).reshape(D, 1)

    h0 = np.zeros((N, D), np.float32)
    h0[:, :ann.shape[1]] = ann
    h0t = np.ascontiguousarray(h0.T)           # [D, N] fp32
    h0t_r = _q12(h0t)
    A_T = np.ascontiguousarray(A.T)            # [2E*N, N]

    # shard layout: core c owns node blocks {128c..128c+127, 1024+128c..+127}
    shard_cols = [np.r_[128 * c:128 * c + 128, 1024 + 128 * c:1024 + 128 * c + 128]
                  for c in range(NC_CORES)]
    h0t_ag = np.ascontiguousarray(np.concatenate(
        [h0t_r[:, shard_cols[c]] for c in range(NC_CORES)], axis=0))

    in_maps = []
    for c in range(NC_CORES):
        in_maps.append({
            "at": np.ascontiguousarray(
                A_T[c * N:(c + 1) * N, :]).astype(np.uint8),
            "h0t": h0t_ag,
            "h0sr": np.ascontiguousarray(h0t_r[:, shard_cols[c]]),
            "h0s": np.ascontiguousarray(h0t[:, shard_cols[c]]),
            "wc": _q12(W_prop[c]),
            "gw": gw_all,
            "bpc": np.ascontiguousarray(b_prop[c].reshape(1, D)),
            "bzc": bz, "brc": br, "bhc": bh,
        })

    return in_maps


def kernel(**inputs):
    from concourse.bass_utils import run_bass_kernel_spmd

    in_maps = prepare_in_maps(
        **{k: inputs[k] for k in ("adjacency", "annotations", "W_prop", "b_prop",
                                  "Wz", "Uz", "bz", "Wr", "Ur", "br",
                                  "Wh", "Uh", "bh")})
    nc = _get_built()
    res = run_bass_kernel_spmd(nc, in_maps, list(range(NC_CORES)), trace=TRACE)
    global LAST_RESULT
    LAST_RESULT = res
    h = np.empty((N, D), np.float32)
    for c in range(NC_CORES):
        sh = res.results[c]["out"].T           # [SH, D] rows in shard order
        h[128 * c:128 * c + 128] = sh[:128]
        h[1024 + 128 * c:1024 + 128 * c + 128] = sh[128:]
    return h



# revision 2
# speedup vs baseline: 1.3287x; 1.3287x over previous
"""GGNN (gated graph NN) message-passing kernel for 8 Trainium2 NeuronCores.

Sharding: edge-type sharding. Core c owns edge-type block c of the adjacency
matrix (stored pre-transposed src-major as [N, N] fp16) plus node shard c
(global node blocks {128c, 1024+128c}) for the GRU update.

Per step, per core c (everything fp16 on the PE, fp32 accumulate in PSUM):
  stage1: t_m = h_m @ W_c per src block m            (h^T gathered as lhsT)
  stage2: partial_a = A_c^T.T @ t, dest halves       (A resident fp16)
  RS1/RS2: ReduceScatter_add fp16 per dest half; RS1 overlaps the second
          half of stage2, RS2 overlaps the first node-half of the GRU
  GRU:    computed in transposed layout [D, SH], split into the two
          128-node halves so half a (from RS1) runs while RS2 is in flight
  AG_a/AG_b: per-half AllGather of h'^T fp16; AG_b overlaps the next
          step's stage1 on the a-half blocks

Stage-1 bias is folded into a host-precomputed per-node constant
abias = sum_e rowdeg(A_e) x b_prop[e], added to a^T after the transpose.
Step 0 exploits h0 = [annotations, 0]: contraction over k<2 only.
"""
import sys
if "/opt/trn_rl_repo" not in sys.path:
    sys.path.insert(0, "/opt/trn_rl_repo")

import numpy as np

NC_CORES = 8
N = 2048          # nodes
D = 512           # state dim
ANN = 256         # annotation dim
STEPS = 5
SH = N // NC_CORES   # 256 nodes per shard
KT = D // 128        # 4
MT = N // 128        # 16


def build(repeats=1, ablate=()):
    import concourse.bacc as bacc
    import concourse.mybir as mybir
    import concourse.tile as tile
    from concourse.masks import make_identity

    dt = mybir.dt
    f16, f32 = dt.float16, dt.float32
    Sig = mybir.ActivationFunctionType.Sigmoid
    Tanh = mybir.ActivationFunctionType.Tanh

    nc = bacc.Bacc()
    at_p = nc.declare_dram_parameter("at", [N, N], f16, isOutput=False)
    # p-major gather layout: row = 128*rank + p, cols = (k, node-in-half)
    h0t_p = nc.declare_dram_parameter("h0t", [NC_CORES * 128, 2, SH], f16,
                                      isOutput=False)
    h0sr_p = nc.declare_dram_parameter("h0sr", [256, SH], f16, isOutput=False)
    h0s_p = nc.declare_dram_parameter("h0s", [256, SH], f32, isOutput=False)
    wc_p = nc.declare_dram_parameter("wc", [D, D], f16, isOutput=False)
    gw_p = nc.declare_dram_parameter("gw", [6, D, D], f16, isOutput=False)
    abt_p = nc.declare_dram_parameter("abt", [D, SH], f32, isOutput=False)
    bz_p = nc.declare_dram_parameter("bzc", [D, 1], f32, isOutput=False)
    br_p = nc.declare_dram_parameter("brc", [D, 1], f32, isOutput=False)
    bh_p = nc.declare_dram_parameter("bhc", [D, 1], f32, isOutput=False)
    out_p = nc.declare_dram_parameter("out", [D, SH], f32, isOutput=True)
    RG = [list(range(NC_CORES))]

    from contextlib import ExitStack
    with tile.TileContext(nc) as tc, ExitStack() as stk:
        res = stk.enter_context(tc.tile_pool(name="res", bufs=1))
        p_mm = stk.enter_context(tc.tile_pool(name="pmm", bufs=8, space="PSUM"))
        p_hc = stk.enter_context(tc.tile_pool(name="phc", bufs=6))
        p_t = stk.enter_context(tc.tile_pool(name="pt", bufs=1))
        p_asb = stk.enter_context(tc.tile_pool(name="pasb", bufs=8))
        p_p1 = stk.enter_context(tc.tile_pool(name="pp1", bufs=1))
        p_an = stk.enter_context(tc.tile_pool(name="pan", bufs=2))
        p_aT = stk.enter_context(tc.tile_pool(name="paT", bufs=2))
        p_g = stk.enter_context(tc.tile_pool(name="pg", bufs=2))
        p_sc = stk.enter_context(tc.tile_pool(name="psc", bufs=4))
        p_h = stk.enter_context(tc.tile_pool(name="ph", bufs=2))
        dram = stk.enter_context(tc.tile_pool(name="dram", bufs=2, space="DRAM"))

        # ---- setup: constants, weights, adjacency (excluded from slope) ----
        identity = res.tile([128, 128], f16, tag="identity")
        make_identity(nc, identity[:])

        bias_tiles = {}
        for nm, par in (("z", bz_p), ("r", br_p), ("h", bh_p)):
            for f in range(KT):
                bt = res.tile([128, 1], f32, tag=f"b{nm}{f}")
                nc.sync.dma_start(bt[:], par[f * 128:(f + 1) * 128, :])
                bias_tiles[(nm, f)] = bt

        abT_t = []
        for k in range(KT):
            ab = res.tile([128, SH], f32, tag=f"abT{k}")
            nc.sync.dma_start(ab[:], abt_p[k * 128:(k + 1) * 128, :])
            abT_t.append(ab)

        wc_t = []
        for k in range(KT):
            w = res.tile([128, D], f16, tag=f"wc{k}")
            nc.sync.dma_start(w[:], wc_p[k * 128:(k + 1) * 128, :])
            wc_t.append(w)

        at_t = []
        for m in range(MT):
            a = res.tile([128, N], f16, tag=f"at{m}")
            nc.sync.dma_start(a[:], at_p[m * 128:(m + 1) * 128, :])
            at_t.append(a)

        gw_res = []
        for g in range(6):
            w = res.tile([128, KT, D], f16, tag=f"gwr{g}")
            nc.scalar.dma_start(w[:], gw_p[g].rearrange("(k p) f -> p k f", p=128))
            gw_res.append(w)

        for rep in range(repeats):
          # step-0 h state: only k=0,1 nonzero (annotations)
          hsh_prev = [None] * KT
          h32_prev = [None] * KT
          for k in range(2):
            hr = p_h.tile([128, SH], f16, tag=f"hsh{k}", name=f"h0r{k}")
            nc.sync.dma_start(hr[:], h0sr_p[k * 128:(k + 1) * 128, :])
            hsh_prev[k] = hr
            h3 = p_h.tile([128, SH], f32, tag=f"h32{k}", name=f"h032{k}")
            nc.sync.dma_start(h3[:], h0s_p[k * 128:(k + 1) * 128, :])
            h32_prev[k] = h3

          ag_prev = None

          for s in range(STEPS):
             last = (s == STEPS - 1)
             kt1 = 2 if s == 0 else KT    # stage1 / U-term contraction depth

             # ---- phase A+B: stage1 (a-half), grp0 partial, stage1 (b-half,
             # deferred so the prev step's AG_b gets ~20us of slack), rest ----
             t_tiles = [None] * MT

             def stage1(m):
                 x, mp = m // 8, m % 8
                 pt = p_mm.tile([128, D], f32, tag="mm", name=f"pt{m}")
                 if "s1" in ablate:
                     nc.tensor.matmul(pt[:], wc_t[0][:, 0:128], wc_t[1][:],
                                      start=True, stop=True)
                 else:
                     hc = p_hc.tile([128, kt1, 128], f16, tag="hc")
                     if s == 0:
                         src = h0t_p[128 * mp:128 * (mp + 1), :,
                                     x * 128:(x + 1) * 128]
                     else:
                         src = ag_prev[x][128 * mp:128 * (mp + 1), :]
                         src = src.rearrange("p (k j) -> p k j", k=kt1)
                     nc.sync.dma_start(hc[:], src)
                     for k in range(kt1):
                         nc.tensor.matmul(pt[:], hc[:, k, :], wc_t[k][:],
                                          start=(k == 0), stop=(k == kt1 - 1))
                 tm = p_t.tile([128, D], f16, tag=f"t{m}")
                 if m % 2 == 0:
                     nc.scalar.copy(tm[:], pt[:])
                 else:
                     nc.vector.tensor_copy(tm[:], pt[:])
                 t_tiles[m] = tm

             # stage2 matmuls for one dest group, a quad of src blocks at a
             # time: 4 consecutive matmuls per PSUM bank (2048 cyc dwell)
             # to avoid HAM oscillation
             def stage2_quads(pas, grp, mq_list, first_m, last_m):
                 for mq in mq_list:
                     for i in range(8):
                         for dm in range(4):
                             m = 4 * mq + dm
                             nc.tensor.matmul(
                                 pas[i][:],
                                 at_t[m][:, grp * 1024 + i * 128:
                                         grp * 1024 + (i + 1) * 128],
                                 t_tiles[m][:],
                                 start=(m == first_m), stop=(m == last_m))

             # grp0 accumulates in two PSUM passes (src 0-7 spilled to SBUF)
             # so stage1 of the b-half runs after grp0's first pass: the
             # prev step's AG_b then has ~20us of slack instead of ~7us.
             part1 = []
             if "s2" in ablate:
                 for m in range(MT):
                     stage1(m)
                 pas0B = [p_mm.tile([128, D], f32, tag="mm", name=f"pa0B{i}")
                          for i in range(8)]
                 pas1 = [p_mm.tile([128, D], f32, tag="mm", name=f"pa1_{i}")
                         for i in range(8)]
                 for i in range(8):
                     t1 = p_p1.tile([128, D], f32, tag=f"p1_{i}", name=f"p1_{i}")
                     nc.vector.memset(t1[:], 0.0)
                     part1.append(t1)
                     nc.tensor.matmul(pas0B[i][:], t_tiles[0][:, 0:128],
                                      t_tiles[1][:], start=True, stop=True)
                     nc.tensor.matmul(pas1[i][:], t_tiles[0][:, 0:128],
                                      t_tiles[1][:], start=True, stop=True)
             else:
                 for m in range(8):
                     stage1(m)
                 pas0A = [p_mm.tile([128, D], f32, tag="mm", name=f"pa0A{i}")
                          for i in range(8)]
                 stage2_quads(pas0A, 0, (0, 1), 0, 7)
                 for i in range(8):
                     t1 = p_p1.tile([128, D], f32, tag=f"p1_{i}", name=f"p1_{i}")
                     if i % 2 == 0:
                         nc.scalar.copy(t1[:], pas0A[i][:])
                     else:
                         nc.vector.tensor_copy(t1[:], pas0A[i][:])
                     part1.append(t1)
                 for m in range(8, MT):
                     stage1(m)
                 pas0B = [p_mm.tile([128, D], f32, tag="mm", name=f"pa0B{i}")
                          for i in range(8)]
                 stage2_quads(pas0B, 0, (2, 3), 8, MT - 1)
                 pas1 = [p_mm.tile([128, D], f32, tag="mm", name=f"pa1_{i}")
                         for i in range(8)]
                 stage2_quads(pas1, 1, (0, 1, 2, 3), 0, MT - 1)

             # ---- RS per dest half ----
             rs_outs = []
             for grp in range(2):
                 rs_in = dram.tile([N // 2, D], f16, tag=f"rs_in{grp}",
                                   name=f"rs_in{grp}")
                 for i in range(8):
                     asb = p_asb.tile([128, D], f16, tag="asb")
                     if grp == 0:
                         nc.vector.tensor_add(asb[:], pas0B[i][:], part1[i][:])
                     else:
                         if i % 2 == 0:
                             nc.scalar.copy(asb[:], pas1[i][:])
                         else:
                             nc.vector.tensor_copy(asb[:], pas1[i][:])
                     eng = nc.sync if i % 2 == 0 else nc.scalar
                     eng.dma_start(rs_in[i * 128:(i + 1) * 128, :], asb[:])
                 rs_out = dram.tile([128, D], f16, tag=f"rs_out{grp}",
                                    name=f"rs_out{grp}")
                 if "cc" in ablate or "rs" in ablate:
                     nc.sync.dma_start(rs_out[:], rs_in[0:128, :])
                 else:
                     nc.gpsimd.collective_compute(
                         "ReduceScatter", mybir.AluOpType.add, replica_groups=RG,
                         ins=[rs_in[:]], outs=[rs_out[:]])
                 rs_outs.append(rs_out)

             # ---- phases D/E: per node-half x — transpose a, GRU, AG ----
             aT = [None] * KT
             h3_new = [None] * KT
             hn_new = [None] * KT
             for x in range(2):
                 xs = slice(x * 128, (x + 1) * 128)
                 an = p_an.tile([128, D], f16, tag="an", name=f"an{x}")
                 nc.gpsimd.dma_start(an[:], rs_outs[x][:])
                 for kb in range(KT):
                     if x == 0:
                         aT[kb] = p_aT.tile([128, SH], f16, tag=f"aT{kb}",
                                            name=f"aT{kb}")
                     ptr = p_mm.tile([128, 128], f16, tag="mm", name=f"ptr{kb}")
                     nc.tensor.transpose(ptr[:], an[:, kb * 128:(kb + 1) * 128],
                                         identity[:])
                     nc.vector.tensor_add(aT[kb][:, xs], ptr[:],
                                          abT_t[kb][:, xs])

                 if "gru" in ablate:
                     pz = [p_mm.tile([128, 128], f32, tag="mm", name=f"pz{f}")
                           for f in range(KT)]
                     pr = [p_mm.tile([128, 128], f32, tag="mm", name=f"pr{f}")
                           for f in range(KT)]
                     ph = [p_mm.tile([128, 128], f32, tag="mm", name=f"phh{f}")
                           for f in range(KT)]
                     for f in range(KT):
                         nc.tensor.matmul(pz[f][:], aT[0][:, 0:128], aT[0][:, xs],
                                          start=True, stop=True)
                         nc.tensor.matmul(pr[f][:], aT[0][:, 0:128], aT[0][:, xs],
                                          start=True, stop=True)
                         nc.tensor.matmul(ph[f][:], aT[0][:, 0:128], aT[0][:, xs],
                                          start=True, stop=True)
                 else:
                     # z and r gates: U-term over h, W-term over aT; sigmoid
                     # emitted right after each gate's stop so PSUM frees fast
                     z_t, r_t = [], []
                     for outs, widx, uidx, nm in ((z_t, 0, 1, "z"),
                                                  (r_t, 2, 3, "r")):
                         pg = [p_mm.tile([128, 128], f32, tag="mm",
                                         name=f"p{nm}{f}") for f in range(KT)]
                         for f in range(KT):
                             for k in range(kt1):
                                 nc.tensor.matmul(
                                     pg[f][:],
                                     gw_res[uidx][:, k, f * 128:(f + 1) * 128],
                                     hsh_prev[k][:, xs],
                                     start=(k == 0), stop=False)
                             for k in range(KT):
                                 nc.tensor.matmul(
                                     pg[f][:],
                                     gw_res[widx][:, k, f * 128:(f + 1) * 128],
                                     aT[k][:, xs],
                                     start=False, stop=(k == KT - 1))
                             og = p_g.tile([128, 128], f32, tag=f"g{nm}{f}",
                                           name=f"g{nm}{f}")
                             nc.scalar.activation(og[:], pg[f][:], Sig,
                                                  bias=bias_tiles[(nm, f)][:])
                             outs.append(og)
                     # h-tilde: W-term prefire, then U-term over r*h
                     ph = [p_mm.tile([128, 128], f32, tag="mm", name=f"phh{f}")
                           for f in range(KT)]
                     for f in range(KT):
                         for k in range(KT):
                             nc.tensor.matmul(
                                 ph[f][:],
                                 gw_res[4][:, k, f * 128:(f + 1) * 128],
                                 aT[k][:, xs], start=(k == 0), stop=False)

                 if "gru" in ablate:
                     z_t, r_t = [], []
                     for outs, pg_l, nm in ((z_t, pz, "z"), (r_t, pr, "r")):
                         for f in range(KT):
                             og = p_g.tile([128, 128], f32, tag=f"g{nm}{f}",
                                           name=f"g{nm}{f}")
                             nc.scalar.activation(og[:], pg_l[f][:], Sig,
                                                  bias=bias_tiles[(nm, f)][:])
                             outs.append(og)

                 rh = []
                 for k in range(kt1):
                     rhk = p_g.tile([128, 128], f16, tag=f"rh{k}", name=f"rh{k}")
                     nc.vector.tensor_mul(rhk[:], r_t[k][:], h32_prev[k][:, xs])
                     rh.append(rhk)
                 if "gru" not in ablate:
                     for f in range(KT):
                         for k in range(kt1):
                             nc.tensor.matmul(
                                 ph[f][:],
                                 gw_res[5][:, k, f * 128:(f + 1) * 128],
                                 rh[k][:], start=False, stop=(k == kt1 - 1))
                 ht_t = []
                 for f in range(KT):
                     og = p_g.tile([128, 128], f32, tag=f"gh{f}", name=f"gh{f}")
                     nc.scalar.activation(og[:], ph[f][:], Tanh,
                                          bias=bias_tiles[("h", f)][:])
                     ht_t.append(og)

                 # ---- h' = h + z * (ht - h);  h'=z*ht where h==0 (s==0,k>=2)
                 if not last and x == 0:
                     ag_in = [dram.tile([128, D], f16, tag=f"ag_in{xx}",
                                        name=f"ag_in{xx}") for xx in range(2)]
                 for k in range(KT):
                     if x == 0:
                         h3_new[k] = p_h.tile([128, SH], f32, tag=f"h32{k}",
                                              name=f"h32n{k}")
                         if not last:
                             hn_new[k] = p_h.tile([128, SH], f16, tag=f"hsh{k}",
                                                  name=f"hshn{k}")
                     h3 = h3_new[k]
                     if s == 0 and k >= 2:
                         if not last:
                             nc.vector.tensor_mul(hn_new[k][:, xs], z_t[k][:],
                                                  ht_t[k][:])
                         nc.vector.tensor_mul(h3[:, xs], z_t[k][:], ht_t[k][:])
                     else:
                         sA = p_sc.tile([128, 128], f32, tag="gsA", name="gsA")
                         nc.vector.tensor_sub(sA[:], ht_t[k][:],
                                              h32_prev[k][:, xs])
                         sB = p_sc.tile([128, 128], f32, tag="gsB", name="gsB")
                         nc.vector.tensor_mul(sB[:], z_t[k][:], sA[:])
                         if not last:
                             nc.vector.tensor_add(hn_new[k][:, xs],
                                                  h32_prev[k][:, xs], sB[:])
                         nc.vector.tensor_add(h3[:, xs], h32_prev[k][:, xs],
                                              sB[:])
                     if last:
                         nc.sync.dma_start(out_p[k * 128:(k + 1) * 128, xs],
                                           h3[:, xs])
                     else:
                         nc.sync.dma_start(ag_in[x][:, k * 128:(k + 1) * 128],
                                           hn_new[k][:, xs])

                 if not last:
                     ag_out = dram.tile([NC_CORES * 128, D], f16,
                                        tag=f"ag_out{x}", name=f"ag_out{x}",
                                        addr_space="Shared")
                     if "cc" in ablate or "ag" in ablate:
                         nc.sync.dma_start(ag_out[0:128, :], ag_in[x][:])
                     else:
                         nc.gpsimd.collective_compute(
                             "AllGather", mybir.AluOpType.bypass,
                             replica_groups=RG,
                             ins=[ag_in[x][:]], outs=[ag_out[:]])
                     if x == 0:
                         ag_next = [ag_out]
                     else:
                         ag_next.append(ag_out)

             if not last:
                 ag_prev = ag_next
                 hsh_prev, h32_prev = hn_new, h3_new

    nc.finalize()
    return nc


_BUILT = None
TRACE = False
LAST_RESULT = None
_BUILT_R = {}


def _get_built(repeats=1, ablate=()):
    global _BUILT
    key = (repeats, tuple(ablate))
    if key != (1, ()):
        if key not in _BUILT_R:
            _BUILT_R[key] = build(repeats, ablate)
        return _BUILT_R[key]
    if _BUILT is None:
        _BUILT = build()
    return _BUILT


def prepare_in_maps(adjacency, annotations, W_prop, b_prop, Wz, Uz, bz,
                    Wr, Ur, br, Wh, Uh, bh):
    A = np.asarray(adjacency, np.float32)
    ann = np.asarray(annotations, np.float32)
    W_prop = np.asarray(W_prop, np.float32)
    b_prop = np.asarray(b_prop, np.float32)
    gw_all = np.stack([np.asarray(w, np.float32)
                       for w in (Wz, Uz, Wr, Ur, Wh, Uh)]).astype(np.float16)
    bz = np.asarray(bz, np.float32).reshape(D, 1)
    br = np.asarray(br, np.float32).reshape(D, 1)
    bh = np.asarray(bh, np.float32).reshape(D, 1)

    h0 = np.zeros((N, D), np.float32)
    h0[:, :ann.shape[1]] = ann
    h0t = np.ascontiguousarray(h0.T)           # [D, N] fp32; rows >=256 are 0
    A_T = np.ascontiguousarray(A.T)            # [2E*N, N] src-major

    # stage-1 bias folded to per-node constant: abias[n] = sum_e deg_e[n]*b_e
    EE = 2 * (A.shape[1] // (2 * N))           # 2E = 8
    deg = A.reshape(N, EE, N).sum(axis=2)      # [N, 2E] row degree per type
    abias = deg @ b_prop                       # [N, D]
    abias_T = np.ascontiguousarray(abias.T)    # [D, N]

    # shard layout: core c owns node blocks {128c..128c+127, 1024+128c..+127}
    shard_cols = [np.r_[128 * c:128 * c + 128, 1024 + 128 * c:1024 + 128 * c + 128]
                  for c in range(NC_CORES)]
    # gather layout for step-0 stage1: p-major [rank*128+d, k, node], k=0,1
    h0t_ag = np.ascontiguousarray(np.concatenate(
        [h0t[0:256, shard_cols[c]].reshape(2, 128, SH).transpose(1, 0, 2)
         for c in range(NC_CORES)], axis=0)).astype(np.float16)

    in_maps = []
    for c in range(NC_CORES):
        in_maps.append({
            "at": np.ascontiguousarray(
                A_T[c * N:(c + 1) * N, :]).astype(np.float16),
            "h0t": h0t_ag,
            "h0sr": np.ascontiguousarray(
                h0t[0:256, shard_cols[c]]).astype(np.float16),
            "h0s": np.ascontiguousarray(h0t[0:256, shard_cols[c]]),
            "wc": W_prop[c].astype(np.float16),
            "gw": gw_all,
            "abt": np.ascontiguousarray(abias_T[:, shard_cols[c]]),
            "bzc": bz, "brc": br, "bhc": bh,
        })
    return in_maps


def kernel(**inputs):
    from concourse.bass_utils import run_bass_kernel_spmd

    in_maps = prepare_in_maps(
        **{k: inputs[k] for k in ("adjacency", "annotations", "W_prop", "b_prop",
                                  "Wz", "Uz", "bz", "Wr", "Ur", "br",
                                  "Wh", "Uh", "bh")})
    nc = _get_built()
    res = run_bass_kernel_spmd(nc, in_maps, list(range(NC_CORES)), trace=TRACE)
    global LAST_RESULT
    LAST_RESULT = res
    h = np.empty((N, D), np.float32)
    for c in range(NC_CORES):
        sh = res.results[c]["out"].T           # [SH, D] rows in shard order
        h[128 * c:128 * c + 128] = sh[:128]
        h[1024 + 128 * c:1024 + 128 * c + 128] = sh[128:]
    return h
